# revision 1
# baseline (speedup 1.0000x reference)
"""Trainium2 Bass kernel v2 for nn_MultiHeadAttention_53017076301867.

Strategy (8 cores, tensor-parallel over H=16 heads, 2 heads/core):
  - Host pre-shards: per-core QKV weight column slices, W_proj row slices,
    x transposed to [E, S] bf16, mask-derived rows, and small host-computed
    local-value summaries (vloc256e / tail / vbar) that replace the entire
    on-device local V projection.
  - Each core computes its 2 heads' global causal attention + the local
    windowed branch + a partial output projection over its 128 ctx features.
  - Host sums the 8 bf16 partial projections + b_proj.

v2 changes vs baseline (224us):
  - local q/k units paired into M=128 matmuls (k units were M=64 half-rate)
  - local v units + vblock removed; vloc256e/tail/vbar host-computed
  - reciprocal -> reciprocal_approx_fast (DVE custom op, ~5x)
  - exp stays on ACT; all PSUM->SBUF copies moved to DVE/GPSIMD
  - rank-1 broadcast matmuls in bf16 (were fp32)
  - xT loaded as 8 separate tiles split across 2 DMA queues (kills the
    15us startup stall waiting on the whole 4MB xT load)
  - output partials in bf16 (halves output DMA)
"""

import numpy as np
import ml_dtypes

S, E, H, WIN, D = 2048, 1024, 16, 256, 64
C = S // WIN            # 8 chunks
NCORES = 8
SCALE = 1.0 / (D ** 0.5)  # 0.125
BF = ml_dtypes.bfloat16
F8 = ml_dtypes.float8_e4m3fn

_prog_cache = {}


def build_program():
    from contextlib import ExitStack
    import concourse.tile as tile
    import concourse.mybir as mybir
    from concourse import bacc
    from concourse.masks import make_identity

    dt = mybir.dt
    f32, bf = dt.float32, dt.bfloat16
    AF = mybir.ActivationFunctionType
    ALU = mybir.AluOpType

    nc = bacc.Bacc("TRN2", target_bir_lowering=False, debug=False)

    xT = nc.dram_tensor("xT", [E, S], bf, kind="ExternalInput").ap()
    f8 = dt.float8e4
    xTl = nc.dram_tensor("xTl", [E, 2 * 128], f8, kind="ExternalInput").ap()
    wq3 = nc.dram_tensor("wq3", [E, 128], bf, kind="ExternalInput").ap()
    wk3 = nc.dram_tensor("wk3", [E, 128], bf, kind="ExternalInput").ap()
    wv3 = nc.dram_tensor("wv3", [E, 128], bf, kind="ExternalInput").ap()
    # host-prearranged [p, i, c, v] so each 4-unit tile is one contiguous DMA
    wlqk = nc.dram_tensor("wlqk", [128, 16, 8, 128], f8, kind="ExternalInput").ap()
    lmask = nc.dram_tensor("lmask", [2, 128, WIN], bf, kind="ExternalInput").ap()
    wpr = nc.dram_tensor("wpr", [128, E], bf, kind="ExternalInput").ap()
    mrow = nc.dram_tensor("mrow", [1, 2, S], f32, kind="ExternalInput").ap()
    wcr = nc.dram_tensor("wcr", [1, S], bf, kind="ExternalInput").ap()
    wbr = nc.dram_tensor("wbr", [1, WIN], f32, kind="ExternalInput").ap()
    v256 = nc.dram_tensor("v256", [128, 2, 2, 65], bf, kind="ExternalInput").ap()
    tailv = nc.dram_tensor("tailv", [1, 2, 65], bf, kind="ExternalInput").ap()
    vbar = nc.dram_tensor("vbar", [1, 2, 64], bf, kind="ExternalInput").ap()
    outp = nc.dram_tensor("outp", [S, E], bf, kind="ExternalOutput").ap()

    with tile.TileContext(nc) as tc, ExitStack() as ctx:
        P = ctx.enter_context(tc.tile_pool(name="persist", bufs=1))

        # ---- input loads: split across 4 queues, phase-1 operands first ----
        wq_sb = P.tile([128, 8, 128], bf)
        nc.sync.dma_start(out=wq_sb, in_=wq3.rearrange("(c p) d -> p c d", p=128))
        wk_sb = P.tile([128, 8, 128], bf)
        nc.scalar.dma_start(out=wk_sb, in_=wk3.rearrange("(c p) d -> p c d", p=128))
        wv_sb = P.tile([128, 8, 128], bf)
        nc.gpsimd.dma_start(out=wv_sb, in_=wv3.rearrange("(c p) d -> p c d", p=128))
        wqkv_sb = [wq_sb, wk_sb, wv_sb]
        xTv = xT.rearrange("(c p) s -> p c s", p=128)
        xTl_sb = P.tile([128, 8, 256], f8)
        nc.scalar.dma_start(out=xTl_sb, in_=xTl.rearrange("(c p) s -> p c s", p=128))
        xT_sb = []
        for ec in range(8):
            t = P.tile([128, S], bf, name=f"xTc{ec}")
            eng = nc.sync if ec % 2 == 0 else nc.scalar
            eng.dma_start(out=t, in_=xTv[:, ec, :])
            xT_sb.append(t)
        wlqk_sb = [P.tile([128, 4, 8, 128], f8, name=f"wlqk{q4}")
                   for q4 in range(4)]
        mrow2_sb = P.tile([1, 2, S], f32)
        nc.gpsimd.dma_start(out=mrow2_sb, in_=mrow)
        wcr_sb = P.tile([1, S], bf)
        nc.gpsimd.dma_start(out=wcr_sb, in_=wcr)
        wbr_sb = P.tile([1, WIN], f32)
        nc.gpsimd.dma_start(out=wbr_sb, in_=wbr)
        lmask_sb = P.tile([128, 2, WIN], bf)
        nc.gpsimd.dma_start(out=lmask_sb, in_=lmask.rearrange("k p w -> p k w"))
        v256_sb = P.tile([128, 2, 2, 65], bf)
        nc.gpsimd.dma_start(out=v256_sb, in_=v256)
        tailv_sb = P.tile([1, 2, 65], bf)
        nc.gpsimd.dma_start(out=tailv_sb, in_=tailv)
        vbar_sb = P.tile([1, 2, 64], bf)
        nc.gpsimd.dma_start(out=vbar_sb, in_=vbar)
        wpr_sb = P.tile([128, E], bf)
        nc.gpsimd.dma_start(out=wpr_sb, in_=wpr)

        ones64b = P.tile([1, 64], bf)
        nc.vector.memset(ones64b, 1.0)
        identb128 = P.tile([128, 128], bf)
        make_identity(nc, identb128)
        onesrow = P.tile([1, WIN], bf)
        nc.vector.memset(onesrow, 1.0)

        QT2 = P.tile([128, S], bf)       # Q^T, rows = 2 heads x 64 dims
        KT2 = P.tile([128, S], bf)
        V2e = P.tile([128, 16, 2, 65], bf)  # [k-part, k-tile, head, d|ones]
        QP = P.tile([64, 2, 8, 16, 16], bf)  # [d, head, u, i, j], perm w~=i*16+j
        KP = P.tile([64, 2, 8, 16, 16], bf)
        bloc = P.tile([128, WIN], f32)
        ctxT = P.tile([128, S], bf)

        # ---------------- phase 1: global QKV projections ----------------
        with tc.tile_pool(name="ps1", bufs=1, space="PSUM") as ps1, \
                tc.tile_pool(name="sb1", bufs=1) as sb1:
            VT2 = sb1.tile([128, S], bf)
            for tsel, dest in ((0, QT2), (1, KT2), (2, VT2)):
                pss = [ps1.tile([128, 512], f32, tag=f"qk{g}", bufs=1,
                                name=f"qkps{g}") for g in range(4)]
                for ec in range(8):
                    for g in range(4):
                        nc.tensor.matmul(
                            pss[g], lhsT=wqkv_sb[tsel][:, ec, :],
                            rhs=xT_sb[ec][:, g * 512:(g + 1) * 512],
                            start=(ec == 0), stop=(ec == 7),
                            skip_group_check=True)
                for g in range(4):
                    nc.scalar.copy(dest[:, g * 512:(g + 1) * 512], pss[g])
            # release the bulk wlqk loads only once the Q projection has
            # drained the xT stream, so they don't steal HBM bandwidth from it
            gate = sb1.tile([1, 4], bf)
            nc.gpsimd.tensor_copy(gate, QT2[0:1, 1536:1540])
            for q4 in range(4):
                nc.gpsimd.dma_start(out=wlqk_sb[q4],
                                    in_=wlqk[:, q4 * 4:(q4 + 1) * 4, :, :])
            for st in range(16):
                pv = ps1.tile([128, 128], bf, tag="vtr", bufs=2)
                nc.tensor.transpose(
                    pv, VT2[:, st * 128:(st + 1) * 128], identb128)
                nc.vector.tensor_copy(
                    V2e[:, st, :, 0:64], pv.rearrange("p (h d) -> p h d", h=2))
            nc.gpsimd.memset(V2e[:, :, :, 64], 1.0)

        # ------- phase 2+3: global attention + paced units/projections ----
        uidx = [0]
        with tc.tile_pool(name="ps3", bufs=2, space="PSUM") as ps3, \
                tc.tile_pool(name="sb3", bufs=4) as sb3:

            def emit_unit():
                if uidx[0] >= 16:
                    return
                i = uidx[0]
                uidx[0] += 1
                ps = ps3.tile([128, 256], f32, tag="aux", bufs=2,
                              name="lqkps")
                for ec in range(8):
                    nc.tensor.matmul(
                        ps, lhsT=wlqk_sb[i // 4][:, i % 4, ec, :],
                        rhs=xTl_sb[:, ec, :],
                        start=(ec == 0), stop=(ec == 7))
                qsrc = ps[0:64, :].rearrange("d (h u j) -> d h u j", h=2, u=8)
                ksrc = ps[64:128, :].rearrange("d (h u j) -> d h u j", h=2, u=8)
                nc.vector.tensor_copy(QP[:, :, :, i, :], qsrc)
                nc.vector.tensor_copy(KP[:, :, :, i, :], ksrc)

            def emit_locattn():
                # local windowed attention: 4 independent score chains
                # (2 heads x 2 key-tiles) emitted back-to-back, then exp/mask,
                # then the AV/tail accumulations and normalization.
                slocs = {}
                ets = {}
                for kt in range(2):
                    sps = ps3.tile([128, 2, 512], f32, tag="sT", bufs=2,
                                   name=f"slocps{kt}")
                    for hh in range(2):
                        for u in range(8):
                            nc.tensor.matmul(
                                sps[:, hh, 0:WIN],
                                lhsT=KP[:, hh, u, kt * 8:(kt + 1) * 8, :],
                                rhs=QP[:, hh, u, :, :],
                                start=(u == 0), stop=(u == 7),
                                skip_group_check=True)
                    slocs[kt] = sps
                for kt in range(2):
                    et = sb3.tile([128, 2, WIN], bf, tag="eloc", bufs=4,
                                  name=f"eloc{kt}")
                    nc.scalar.activation(et, slocs[kt][:, :, 0:WIN], AF.Exp,
                                         scale=SCALE / C)
                    for hh in range(2):
                        nc.vector.tensor_mul(et[:, hh, :], et[:, hh, :],
                                             lmask_sb[:, kt, :])
                        ets[(kt, hh)] = et[:, hh, :]
                plocs = {}
                for hh in range(2):
                    ploc = ps3.tile([65, WIN], f32, tag="aux", bufs=2,
                                    name=f"plocps{hh}")
                    for kt in range(2):
                        nc.tensor.matmul(ploc, lhsT=v256_sb[:, hh, kt, :],
                                         rhs=ets[(kt, hh)], start=(kt == 0),
                                         stop=False, skip_group_check=True)
                    nc.tensor.matmul(ploc, lhsT=tailv_sb[:, hh, :],
                                     rhs=onesrow, start=False, stop=True,
                                     skip_group_check=True)
                    plocs[hh] = ploc
                for hh in range(2):
                    ploc = plocs[hh]
                    zl = sb3.tile([1, WIN], f32, tag="zl")
                    zsl = sb3.tile([1, WIN], f32, tag="zsl")
                    nc.scalar.copy(zsl, ploc[64:65, :])
                    nc.vector.reciprocal_approx_fast(zl, zsl)
                    rbl = sb3.tile([1, WIN], f32, tag="rbl")
                    nc.vector.tensor_mul(rbl, zl, wbr_sb)
                    rbls = sb3.tile([64, WIN], f32, tag="rbls")
                    nc.gpsimd.partition_broadcast(rbls, rbl)
                    nc.vector.tensor_mul(bloc[hh * 64:(hh + 1) * 64, :],
                                         ploc[0:64, :], rbls)

            # -- global causal attention, software-pipelined --
            projq = []

            def pace(step_no):
                # 16 qk units spread over first 26 steps (done by end of g=2),
                # then one deferred projection per step; the step offset gives
                # each blend's vector chain time to drain before its
                # projections hit the in-order tensor queue
                want = min(16, (step_no * 16 + 25) // 26 + 1)
                while uidx[0] < want:
                    emit_unit()
                if uidx[0] >= 16 and projq and step_no >= projq[0][1]:
                    emit_proj(projq.pop(0)[0])

            pjn = [0]

            def emit_proj(qt, tail=False):
                for half in range(2):
                    tag = ("sT" if (tail and half == 0) else "aux")
                    pp = ps3.tile([128, 512], f32, tag=tag,
                                  bufs=2, name="ppps")
                    nc.tensor.matmul(
                        pp, lhsT=ctxT[:, qt * 128:(qt + 1) * 128],
                        rhs=wpr_sb[:, half * 512:(half + 1) * 512],
                        start=True, stop=True)
                    ot = sb3.tile([128, 512], bf, tag="ot", bufs=3)
                    pjn[0] += 1
                    if pjn[0] % 2:
                        nc.vector.tensor_copy(ot, pp)
                    else:
                        nc.scalar.copy(ot, pp)
                    dq = nc.sync if pjn[0] % 2 else nc.scalar
                    dq.dma_start(
                        out=outp[qt * 128:(qt + 1) * 128,
                                 half * 512:(half + 1) * 512],
                        in_=ot)

            step = [0]

            def make_blend(g, gpss):
                def blend():
                    zrow = sb3.tile([1, 2, 512], f32, tag="zrow", bufs=2)
                    zsr = sb3.tile([1, 2, 512], f32, tag="zsr", bufs=2)
                    nc.scalar.copy(zsr, gpss[64:65, :, :])
                    nc.vector.reciprocal_approx_fast(zrow, zsr)
                    ra = sb3.tile([1, 2, 512], f32, tag="ra", bufs=2)
                    nc.vector.tensor_mul(
                        ra, zrow, mrow2_sb[:, :, g * 512:(g + 1) * 512])
                    rbs = sb3.tile([64, 2, 512], f32, tag="rbs", bufs=2)
                    nc.gpsimd.partition_broadcast(rbs, ra)
                    for hh in range(2):
                        hs = slice(hh * 64, hh * 64 + 64)
                        cps = ps3.tile([64, 512], f32, tag="aux", bufs=2,
                                       name="cpps")
                        nc.tensor.matmul(
                            cps, lhsT=vbar_sb[:, hh, :],
                            rhs=wcr_sb[:, g * 512:(g + 1) * 512],
                            start=True, stop=True)
                        dst = ctxT[hs, g * 512:(g + 1) * 512]
                        nc.vector.tensor_mul(dst, gpss[0:64, hh, :], rbs[:, hh, :])
                        nc.vector.tensor_add(dst, dst, cps)
                    for qt in range(4 * g + 3, 4 * g - 1, -1):
                        if qt >= 2:
                            projq.append((qt, step[0] + 3))
                return blend

            # descending g: by end of g=1 all units have drained (locattn dep)
            for g in (3, 2, 1, 0):
                gpss = ps3.tile([65, 2, 512], f32, tag="g01", bufs=1,
                                name="gctxps")
                nkt = 4 * g + 4
                pend = []
                for t in range(nkt + 2):
                    if t < nkt:
                        # boundary tiles: only q-columns >= k are live
                        off = (t % 4) * 128 if t // 4 == g else 0
                        sps = ps3.tile([128, 2, 512], f32, tag="sT", bufs=2,
                                       name="sTps")
                        for hh in range(2):
                            hs = slice(hh * 64, hh * 64 + 64)
                            nc.tensor.matmul(
                                sps[:, hh, off:],
                                lhsT=KT2[hs, t * 128:(t + 1) * 128],
                                rhs=QT2[hs, g * 512 + off:(g + 1) * 512],
                                start=True, stop=True, skip_group_check=True)
                        et = sb3.tile([128, 2, 512], bf, tag="expT", bufs=6)
                        nc.scalar.activation(et[:, :, off:], sps[:, :, off:],
                                             AF.Exp, scale=SCALE)
                        if t >= 4 * g:
                            nc.gpsimd.affine_select(
                                et[:, :, off:], et[:, :, off:],
                                pattern=[[0, 2], [1, 512 - off]],
                                base=0, channel_multiplier=-1,
                                compare_op=ALU.is_ge, fill=0.0)
                        pend.append((t, et, off))
                        pace(step[0])
                        step[0] += 1
                    if len(pend) > 2 or (t >= nkt and pend):
                        pt, pet, poff = pend.pop(0)
                        for hh in range(2):
                            nc.tensor.matmul(
                                gpss[:, hh, poff:], lhsT=V2e[:, pt, hh, :],
                                rhs=pet[:, hh, poff:],
                                start=(pt == 0), stop=(pt == nkt - 1),
                                skip_group_check=True)
                make_blend(g, gpss)()
                if g == 2:
                    emit_locattn()
            for hh in range(2):
                hs = slice(hh * 64, hh * 64 + 64)
                dperm = ctxT[hs, 0:WIN].rearrange("p (j i) -> p i j", j=16)
                nc.vector.tensor_add(dperm, dperm, bloc[hs, :])
            while projq:
                emit_proj(projq.pop(0)[0], tail=True)
            for qt in (1, 0):
                emit_proj(qt, tail=True)

    nc.compile()
    return nc


def prep_inputs(x, global_attention_mask, W_local_query, W_local_key,
                W_local_value, W_query, W_key, W_value, W_proj):
    """Host-side sharding/layout prep. Returns list of per-core input dicts."""
    def b(a):
        return np.ascontiguousarray(np.asarray(a, np.float32)).astype(BF)

    x2 = np.asarray(x, np.float32).reshape(S, E)
    xT_np = np.ascontiguousarray(x2.T).astype(BF)                   # [E, S]
    # per-i interleave [q_i | k_i] for paired local projections
    Wlq = np.asarray(W_local_query, np.float32).reshape(E, 16, 64)
    Wlk = np.asarray(W_local_key, np.float32).reshape(E, 16, 64)
    Wlv = np.asarray(W_local_value, np.float32)
    wlqk_e = np.concatenate([Wlq, Wlk], axis=2)               # [E, 16, 128]
    # rearrange to [p, i, c, v] (E = c*128 + p) for contiguous tile DMAs
    wlqk_np = np.ascontiguousarray(
        wlqk_e.reshape(8, 128, 16, 128).transpose(1, 2, 0, 3)).astype(F8)
    # local causal mask in permuted order w~ = i*16 + j (true w = j*16 + i)
    wt = np.arange(WIN)
    w_of = (wt % 16) * 16 + wt // 16
    lmask_np = np.ascontiguousarray(
        (w_of.reshape(2, 128)[:, :, None] <= w_of[None, None, :])
        .astype(np.float32)).astype(BF)                       # [2, 128, WIN]
    m = np.asarray(global_attention_mask, np.float32).reshape(S)
    q = np.arange(S)
    mrow_np = np.ascontiguousarray(
        np.broadcast_to(m.reshape(1, 1, S), (1, 2, S)))
    wcr_np = b(((1.0 - m) * (q >= WIN) / S).reshape(1, S))
    wbr_np = np.ascontiguousarray(((1.0 - m)[w_of]).reshape(1, WIN)
                                  ).astype(np.float32)        # permuted order
    Wq = np.asarray(W_query, np.float32)
    Wk = np.asarray(W_key, np.float32)
    Wv = np.asarray(W_value, np.float32)
    Wp = np.asarray(W_proj, np.float32)

    # ---- host-computed local-value summaries ----
    # vloc[h,k,d] = vl[h*128 + k//16, (k%16)*64 + d], vl = x @ Wlv
    # Individual rows needed: k<WIN -> vl rows h*128 + r for r<16 (16/head).
    # Sums: colsum over full block and first-16 rows via xsum @ Wlv.
    rows16 = (np.arange(H)[:, None] * 128 + np.arange(16)[None, :]).ravel()
    vl16 = x2[rows16] @ Wlv                       # [H*16, E] rows r<16
    vl16 = vl16.reshape(H, 16, E)
    xsumA = x2.reshape(H, 128, E).sum(axis=1)     # [H, E] full-block sums
    colsumA = xsumA @ Wlv                         # [H, E]
    colsum16 = vl16.sum(axis=1)                   # [H, E]
    jj = np.arange(16)
    # vbar[h,d] = sum_jj colsumA[h, jj*64+d]  (wcr carries the 1/S)
    vbarH = colsumA.reshape(H, 16, 64).sum(axis=1)           # [H, 64]
    tailH = (colsumA - colsum16).reshape(H, 16, 64).sum(axis=1)  # [H, 64]

    in_maps = []
    for i in range(NCORES):
        cs = slice(i * 128, (i + 1) * 128)
        v256_np = np.zeros((128, 2, 2, 65), np.float32)
        tail_np = np.zeros((1, 2, 65), np.float32)
        vbar_np = np.zeros((1, 2, 64), np.float32)
        for hh in range(2):
            hg = 2 * i + hh
            for kt in range(2):
                wt_ = kt * 128 + np.arange(128)          # permuted key idx
                k_true = 16 * (wt_ % 16) + wt_ // 16
                # vloc[hg, k_true, d] = vl[hg*128 + k_true//16,
                #                          (k_true%16)*64 + d]
                r = k_true // 16                          # < 16
                cpos = k_true % 16
                v256_np[:, hh, kt, 0:64] = vl16[
                    hg, r][np.arange(128)[:, None],
                           (cpos * 64)[:, None] + np.arange(64)[None, :]]
            v256_np[:, hh, :, 64] = 1.0
            tail_np[0, hh, 0:64] = tailH[hg]
            tail_np[0, hh, 64] = S - WIN
            vbar_np[0, hh, :] = vbarH[hg]
        in_maps.append({
            "xT": xT_np,
            "xTl": np.ascontiguousarray(
                x2.T[:, i * 256:(i + 1) * 256]).astype(F8),
            "wq3": b(Wq[:, cs]),
            "wk3": b(Wk[:, cs]),
            "wv3": b(Wv[:, cs]),
            "wlqk": wlqk_np,
            "lmask": lmask_np,
            "wpr": b(Wp[cs, :]),
            "mrow": mrow_np,
            "wcr": wcr_np,
            "wbr": wbr_np,
            "v256": v256_np.astype(BF),
            "tailv": tail_np.astype(BF),
            "vbar": vbar_np.astype(BF),
        })
    return in_maps


def kernel(x, global_attention_mask, W_local_query, W_local_key, W_local_value,
           W_query, W_key, W_value, W_proj, b_proj):
    from concourse.bass_utils import run_bass_kernel_spmd

    if "nc" not in _prog_cache:
        _prog_cache["nc"] = build_program()
    nc = _prog_cache["nc"]

    in_maps = prep_inputs(x, global_attention_mask, W_local_query, W_local_key,
                          W_local_value, W_query, W_key, W_value, W_proj)
    res = run_bass_kernel_spmd(nc, in_maps, core_ids=list(range(NCORES)))
    out = np.zeros((S, E), np.float32)
    for r in res.results:
        out += np.asarray(r["outp"], np.float32)
    out = out + np.asarray(b_proj, np.float32)[None, :]
    return out[None].astype(np.float32)



# revision 7
# speedup vs baseline: 1.1465x; 1.1465x over previous
"""Trainium2 Bass kernel v3 for nn_MultiHeadAttention_53017076301867.

Strategy (8 cores, tensor-parallel over H=16 heads, 2 heads/core):
  - The reference blends ctx = mask ? global_attn : local_attn per row.
    The device computes the global branch ONLY for the gathered (sorted)
    mask==1 query positions; causal masking of gathered queries vs key
    tiles uses per-key-partition cutoff columns (DVE tensor_mask custom
    op) computed on the host.
  - Local windowed branch (all q<WIN rows, permuted order) via fp8
    paired q/k unit projections (as v2); projected directly; the host
    picks the mask==0 & q<WIN rows.
  - mask==0 & q>=WIN rows are one constant row (uniform local average),
    computed on the host.
  - Each core also computes its 128-feature slice of the output
    projection for the gathered + local rows; host sums 8 bf16 partials,
    scatters rows, and adds b_proj.
  - PE warm-up: junk matmuls during the initial DMA window keep the HAM
    clock gate at 8/8 for the real work.
"""

import numpy as np
import ml_dtypes

S, E, H, WIN, D = 2048, 1024, 16, 256, 64
C = S // WIN            # 8 chunks
NCORES = 8
SCALE = 1.0 / (D ** 0.5)  # 0.125
BF = ml_dtypes.bfloat16
F8 = ml_dtypes.float8_e4m3fn

_prog_cache = {}
FLAGS = dict(mask=True, iota=True, warm=True, batched_out=True)


def make_plan(mask):
    m = np.asarray(mask, np.int64).reshape(S)
    gidx = np.where(m == 1)[0]
    n1 = len(gidx)
    widths = []
    r = n1
    while r > 512:
        widths.append(512)
        r -= 512
    widths.append(max(128, -(-r // 128) * 128))
    n1p = sum(widths)
    gpad = np.concatenate([gidx, np.full(n1p - n1, gidx[-1], np.int64)])
    blocks = []
    off = 0
    for w in widths:
        pb = gpad[off:off + w]
        kt = int(pb.max()) // 128 + 1
        tiles = []
        for t in range(kt):
            c_t = int(np.searchsorted(pb, t * 128))
            cut = np.searchsorted(pb, t * 128 + np.arange(128)).astype(np.int64)
            nm = bool((cut > c_t).any())
            tiles.append(dict(t=t, ca=c_t & ~3, cut=cut, nm=nm, mcol=-1))
        blocks.append(dict(off=off, w=w, kt=kt, tiles=tiles))
        off += w
    order = sorted(range(len(blocks)),
                   key=lambda b: (-blocks[b]['w'], -blocks[b]['kt']))
    nt = 0
    roff = 0
    for b in order:
        bl = blocks[b]
        bl['roff'] = roff
        roff += bl['w']
        for ti in bl['tiles']:
            if ti['nm']:
                ti['mcol'] = nt
                nt += 1
    qgw = []
    r = n1p
    while r > 0:
        qgw.append(min(512, r))
        r -= qgw[-1]
    return dict(n1=n1, n1p=n1p, widths=widths, gpad=gpad, blocks=blocks,
                order=order, nt=max(nt, 1), qgw=qgw, nrows=n1p + WIN)


def plan_key(plan):
    parts = [tuple(plan['widths'])]
    for b in plan['order']:
        bl = plan['blocks'][b]
        parts.append((bl['off'], bl['w'], bl['kt'], bl['roff'],
                      tuple((ti['t'], ti['ca'], ti['nm'], ti['mcol'])
                            for ti in bl['tiles'])))
    return tuple(parts)


def build_program(plan):
    from contextlib import ExitStack
    import concourse.tile as tile
    import concourse.mybir as mybir
    from concourse import bacc
    from concourse.masks import make_identity
    from concourse.dve_ops import TENSOR_MASK

    dt = mybir.dt
    f32, bf, f8 = dt.float32, dt.bfloat16, dt.float8e4
    AF = mybir.ActivationFunctionType

    n1p = plan['n1p']
    NT = plan['nt']
    NR = plan['nrows']

    nc = bacc.Bacc("TRN2", target_bir_lowering=False, debug=False)

    xT = nc.dram_tensor("xT", [128, 8, S], bf, kind="ExternalInput").ap()
    xgT = nc.dram_tensor("xgT", [128, 8, n1p], bf, kind="ExternalInput").ap()
    wqkv = nc.dram_tensor("wqkv", [128, 3, 8, 128], bf, kind="ExternalInput").ap()
    xTl = nc.dram_tensor("xTl", [128, 8, 2 * 128], f8, kind="ExternalInput").ap()
    wlqk = nc.dram_tensor("wlqk", [128, 16, 8, 128], f8, kind="ExternalInput").ap()
    lmask = nc.dram_tensor("lmask", [128, 2, WIN], bf, kind="ExternalInput").ap()
    wpr = nc.dram_tensor("wpr", [128, E], bf, kind="ExternalInput").ap()
    v256 = nc.dram_tensor("v256", [128, 2, 2, 65], bf, kind="ExternalInput").ap()
    tailv = nc.dram_tensor("tailv", [1, 2, 65], bf, kind="ExternalInput").ap()
    cuts = nc.dram_tensor("cuts", [128, NT], f32, kind="ExternalInput").ap()
    outp = nc.dram_tensor("outp", [NR, E], bf, kind="ExternalOutput").ap()
    outv = outp.rearrange("(r p) e -> p r e", p=128)

    with tile.TileContext(nc) as tc, ExitStack() as ctx:
        P = ctx.enter_context(tc.tile_pool(name="persist", bufs=1))

        # ---------------- input DMAs (order = per-queue priority) --------
        xgT_sb = P.tile([128, 8, n1p], bf)
        nc.sync.dma_start(out=xgT_sb[:, 0:4, :], in_=xgT[:, 0:4, :])
        nc.sync.dma_start(out=xgT_sb[:, 4:8, :], in_=xgT[:, 4:8, :])
        xT_sb = P.tile([128, 8, S], bf)
        nc.sync.dma_start(out=xT_sb[:, 0:2, :], in_=xT[:, 0:2, :])
        nc.sync.dma_start(out=xT_sb[:, 2:4, :], in_=xT[:, 2:4, :])
        xTl_sb = P.tile([128, 8, 256], f8)
        nc.scalar.dma_start(out=xTl_sb, in_=xTl)
        wqkv_sb = P.tile([128, 3, 8, 128], bf)
        nc.scalar.dma_start(out=wqkv_sb, in_=wqkv)
        nc.scalar.dma_start(out=xT_sb[:, 4:6, :], in_=xT[:, 4:6, :])
        nc.scalar.dma_start(out=xT_sb[:, 6:8, :], in_=xT[:, 6:8, :])
        wlqk_sb = P.tile([128, 16, 8, 128], f8)
        for q4 in range(4):
            nc.gpsimd.dma_start(out=wlqk_sb[:, 4 * q4:4 * q4 + 4, :, :],
                                in_=wlqk[:, 4 * q4:4 * q4 + 4, :, :])
        lmask_sb = P.tile([128, 2, WIN], bf)
        nc.gpsimd.dma_start(out=lmask_sb, in_=lmask)
        v256_sb = P.tile([128, 2, 2, 65], bf)
        nc.gpsimd.dma_start(out=v256_sb, in_=v256)
        tailv_sb = P.tile([1, 2, 65], bf)
        nc.gpsimd.dma_start(out=tailv_sb, in_=tailv)
        wpr_sb = P.tile([128, E], bf)
        nc.gpsimd.dma_start(out=wpr_sb, in_=wpr)
        cuts_sb = P.tile([128, NT], f32)
        nc.gpsimd.dma_start(out=cuts_sb, in_=cuts)

        # ---------------- setup ----------------
        warm = P.tile([128, 512], bf)
        nc.vector.memset(warm, 0.125)
        onesrow = P.tile([1, WIN], bf)
        nc.vector.memset(onesrow, 1.0)
        identb128 = P.tile([128, 128], bf)
        make_identity(nc, identb128)
        negio = P.tile([128, 512], f32)
        if FLAGS['iota']:
            nc.gpsimd.iota(negio, pattern=[[-1, 512]], base=0,
                           channel_multiplier=0,
                           allow_small_or_imprecise_dtypes=True)
        else:
            nc.vector.memset(negio, 0.0)

        QT2g = P.tile([128, n1p], bf)
        KT2 = P.tile([128, S], bf)
        V2e = P.tile([128, 16, 2, 65], bf)
        QP = P.tile([64, 2, 8, 16, 16], bf)
        KP = P.tile([64, 2, 8, 16, 16], bf)
        ctxT = P.tile([128, n1p], bf)
        blocb = P.tile([128, WIN], bf)

        # ---------------- phase 1: projections ----------------
        with tc.tile_pool(name="ps1", bufs=1, space="PSUM") as ps1, \
                tc.tile_pool(name="sb1", bufs=1) as sb1:
            # HAM warm-up junk matmuls (never read)
            if FLAGS['warm']:
                wps = ps1.tile([128, 512], f32, tag="qg", bufs=2,
                               name="warmps")
                for _ in range(16):
                    nc.tensor.matmul(wps, lhsT=warm[:, 0:128], rhs=warm,
                                     start=True, stop=True,
                                     skip_group_check=True)
            # local q/k units (fp8): QP/KP[d, h, u(chunk), i, j]
            for i in range(16):
                ps = ps1.tile([128, 256], f32, tag="u", bufs=2, name="ups")
                for ec in range(8):
                    nc.tensor.matmul(ps, lhsT=wlqk_sb[:, i, ec, :],
                                     rhs=xTl_sb[:, ec, :],
                                     start=(ec == 0), stop=(ec == 7))
                qsrc = ps[0:64, :].rearrange("d (h u j) -> d h u j", h=2, u=8)
                ksrc = ps[64:128, :].rearrange("d (h u j) -> d h u j", h=2, u=8)
                nc.vector.tensor_copy(QP[:, :, :, i, :], qsrc)
                nc.vector.tensor_copy(KP[:, :, :, i, :], ksrc)
            # gathered-Q projection
            qoff = 0
            for gw in plan['qgw']:
                ps = ps1.tile([128, 512], f32, tag="qg", bufs=2, name="qgps")
                for ec in range(8):
                    nc.tensor.matmul(ps[:, 0:gw], lhsT=wqkv_sb[:, 0, ec, :],
                                     rhs=xgT_sb[:, ec, qoff:qoff + gw],
                                     start=(ec == 0), stop=(ec == 7))
                nc.vector.tensor_copy(QT2g[:, qoff:qoff + gw], ps[:, 0:gw])
                qoff += gw
            # K and V projections (ec-outer to track chunk arrival)
            VT2 = sb1.tile([128, S], bf)
            for tsel, dest in ((1, KT2), (2, VT2)):
                pss = [ps1.tile([128, 512], f32, tag=f"kv{g}", bufs=1,
                                name=f"kvps{g}") for g in range(4)]
                for ec in range(8):
                    for g in range(4):
                        nc.tensor.matmul(
                            pss[g], lhsT=wqkv_sb[:, tsel, ec, :],
                            rhs=xT_sb[:, ec, g * 512:(g + 1) * 512],
                            start=(ec == 0), stop=(ec == 7),
                            skip_group_check=True)
                for g in range(4):
                    nc.vector.tensor_copy(dest[:, g * 512:(g + 1) * 512],
                                          pss[g])
            for st in range(16):
                pv = ps1.tile([128, 128], bf, tag="u", bufs=2, name="pvps")
                nc.tensor.transpose(pv, VT2[:, st * 128:(st + 1) * 128],
                                    identb128)
                nc.vector.tensor_copy(V2e[:, st, :, 0:64],
                                      pv.rearrange("p (h d) -> p h d", h=2))
            nc.gpsimd.memset(V2e[:, :, :, 64], 1.0)

        # ---------------- phase 2: attention + projections ----------------
        with tc.tile_pool(name="ps3", bufs=2, space="PSUM") as ps3, \
                tc.tile_pool(name="sb3", bufs=4) as sb3:
            # ---- local windowed attention ----
            slocs = {}
            ets = {}
            for k2 in range(2):
                sps = ps3.tile([128, 2, 512], f32, tag="sT", bufs=2,
                               name=f"slocps{k2}")
                for hh in range(2):
                    for u in range(8):
                        nc.tensor.matmul(
                            sps[:, hh, 0:WIN],
                            lhsT=KP[:, hh, u, k2 * 8:(k2 + 1) * 8, :],
                            rhs=QP[:, hh, u, :, :],
                            start=(u == 0), stop=(u == 7),
                            skip_group_check=True)
                slocs[k2] = sps
            for k2 in range(2):
                et = sb3.tile([128, 2, WIN], bf, tag="eloc", bufs=2,
                              name=f"eloc{k2}")
                nc.scalar.activation(et, slocs[k2][:, :, 0:WIN], AF.Exp,
                                     scale=SCALE / C)
                for hh in range(2):
                    nc.vector.tensor_mul(et[:, hh, :], et[:, hh, :],
                                         lmask_sb[:, k2, :])
                    ets[(k2, hh)] = et[:, hh, :]
            for hh in range(2):
                ploc = ps3.tile([65, WIN], f32, tag="aux", bufs=2,
                                name=f"plocps{hh}")
                for k2 in range(2):
                    nc.tensor.matmul(ploc, lhsT=v256_sb[:, hh, k2, :],
                                     rhs=ets[(k2, hh)], start=(k2 == 0),
                                     stop=False, skip_group_check=True)
                nc.tensor.matmul(ploc, lhsT=tailv_sb[:, hh, :],
                                 rhs=onesrow, start=False, stop=True,
                                 skip_group_check=True)
                zsl = sb3.tile([1, WIN], f32, tag="zsl", bufs=2)
                nc.vector.tensor_copy(zsl, ploc[64:65, :])
                zl = sb3.tile([1, WIN], f32, tag="zl", bufs=2)
                nc.vector.reciprocal_approx_fast(zl, zsl)
                rbls = sb3.tile([64, WIN], f32, tag="rbls", bufs=2)
                nc.gpsimd.partition_broadcast(rbls, zl)
                nc.vector.tensor_mul(blocb[hh * 64:(hh + 1) * 64, :],
                                     ploc[0:64, :], rbls)

            def emit_proj(colbase, roff, nqt, src, bname):
                stg = sb3.tile([128, nqt, E], bf, tag=f"stg{bname}", bufs=1,
                               name=f"stg{bname}")
                for j in range(nqt):
                    for half in range(2):
                        pp = ps3.tile([128, 512], f32, tag="aux", bufs=2,
                                      name="ppps")
                        nc.tensor.matmul(
                            pp, lhsT=src[:, colbase + j * 128:
                                         colbase + (j + 1) * 128],
                            rhs=wpr_sb[:, half * 512:(half + 1) * 512],
                            start=True, stop=True)
                        nc.vector.tensor_copy(
                            stg[:, j, half * 512:(half + 1) * 512], pp)
                if FLAGS['batched_out']:
                    nc.sync.dma_start(
                        out=outv[:, roff // 128: roff // 128 + nqt, :],
                        in_=stg)
                else:
                    for j in range(nqt):
                        nc.sync.dma_start(
                            out=outp[roff + j * 128: roff + (j + 1) * 128, :],
                            in_=stg[:, j, :])

            # local rows projected early (data ready; output rows at n1p)
            emit_proj(0, n1p, 2, blocb, "loc")

            # ---- gathered global attention blocks ----
            def do_block(bl):
                w, off, kt = bl['w'], bl['off'], bl['kt']
                pack = 512 // w
                tiles = bl['tiles']
                ngrp = -(-len(tiles) // pack)
                gpss = ps3.tile([65, 2, 512], f32, tag="g01", bufs=1,
                                name="gctxps")
                pend = []
                for gi in range(ngrp + 3):
                    if gi < ngrp:
                        grp = tiles[gi * pack:(gi + 1) * pack]
                        ca0 = grp[0]['ca'] if pack == 1 else 0
                        sps = ps3.tile([128, 2, 512], f32, tag="sT", bufs=2,
                                       name="sTps")
                        for qi, ti in enumerate(grp):
                            t, ca = ti['t'], ti['ca']
                            for hh in range(2):
                                hs = slice(hh * 64, hh * 64 + 64)
                                nc.tensor.matmul(
                                    sps[:, hh, qi * w + ca:(qi + 1) * w],
                                    lhsT=KT2[hs, t * 128:(t + 1) * 128],
                                    rhs=QT2g[hs, off + ca:off + w],
                                    start=(qi == 0), stop=(qi == len(grp) - 1),
                                    skip_group_check=True)
                        et = sb3.tile([128, 2, 512], bf, tag="expT", bufs=6,
                                      name="etT")
                        gw = len(grp) * w
                        nc.scalar.activation(et[:, :, ca0:gw],
                                             sps[:, :, ca0:gw], AF.Exp,
                                             scale=SCALE)
                        for qi, ti in enumerate(grp):
                            if ti['nm'] and FLAGS['mask']:
                                ca = ti['ca']
                                for hh in range(2):
                                    sl = (slice(None), hh,
                                          slice(qi * w + ca, (qi + 1) * w))
                                    nc.vector._custom_dve(
                                        TENSOR_MASK,
                                        out=et[sl], in0=et[sl],
                                        in1=negio[:, ca:w],
                                        s0=cuts_sb[:, ti['mcol']:
                                                   ti['mcol'] + 1],
                                        imm2=0.0)
                        pend.append((grp, et))
                    if len(pend) > 2 or (gi >= ngrp and pend):
                        pgrp, pet = pend.pop(0)
                        for qi, ti in enumerate(pgrp):
                            t, ca = ti['t'], ti['ca']
                            for hh in range(2):
                                nc.tensor.matmul(
                                    gpss[:, hh, ca:w],
                                    lhsT=V2e[:, t, hh, :],
                                    rhs=pet[:, hh, qi * w + ca:(qi + 1) * w],
                                    start=(t == 0), stop=(t == kt - 1),
                                    skip_group_check=True)
                # blend: ctxT = gpss[0:64] / gpss[64]
                zsr = sb3.tile([1, 2, w], f32, tag=f"zsr{w}", bufs=2)
                nc.vector.tensor_copy(zsr, gpss[64:65, :, 0:w])
                zrow = sb3.tile([1, 2, w], f32, tag=f"zrow{w}", bufs=2)
                nc.vector.reciprocal_approx_fast(zrow, zsr)
                rbs = sb3.tile([64, 2, w], f32, tag=f"rbs{w}", bufs=2)
                nc.gpsimd.partition_broadcast(rbs, zrow)
                for hh in range(2):
                    hs = slice(hh * 64, hh * 64 + 64)
                    nc.vector.tensor_mul(ctxT[hs, off:off + w],
                                         gpss[0:64, hh, 0:w], rbs[:, hh, :])

            prev = None
            for b in plan['order']:
                do_block(plan['blocks'][b])
                if prev is not None:
                    bl = plan['blocks'][prev]
                    emit_proj(bl['off'], bl['roff'], bl['w'] // 128, ctxT,
                              str(prev))
                prev = b
            bl = plan['blocks'][prev]
            emit_proj(bl['off'], bl['roff'], bl['w'] // 128, ctxT, str(prev))

    nc.compile()
    return nc


def prep_inputs(x, global_attention_mask, W_local_query, W_local_key,
                W_local_value, W_query, W_key, W_value, W_proj, plan=None):
    """Host-side sharding/layout prep. Returns (plan, per-core input dicts,
    const_ctx_row[E])."""
    if plan is None:
        plan = make_plan(global_attention_mask)

    def b(a):
        return np.ascontiguousarray(np.asarray(a, np.float32)).astype(BF)

    x2 = np.asarray(x, np.float32).reshape(S, E)
    xT_np = np.ascontiguousarray(
        x2.T.reshape(8, 128, S).transpose(1, 0, 2)).astype(BF)
    xg = x2[plan['gpad']]
    xgT_np = np.ascontiguousarray(
        xg.T.reshape(8, 128, plan['n1p']).transpose(1, 0, 2)).astype(BF)

    Wq = np.asarray(W_query, np.float32)
    Wk = np.asarray(W_key, np.float32)
    Wv = np.asarray(W_value, np.float32)
    Wp = np.asarray(W_proj, np.float32)
    Wlv = np.asarray(W_local_value, np.float32)

    # local unit weights: per-i interleave [q_i | k_i], [p, i, c, v]
    Wlq = np.asarray(W_local_query, np.float32).reshape(E, 16, 64)
    Wlk = np.asarray(W_local_key, np.float32).reshape(E, 16, 64)
    wlqk_e = np.concatenate([Wlq, Wlk], axis=2)               # [E, 16, 128]
    wlqk_np = np.ascontiguousarray(
        wlqk_e.reshape(8, 128, 16, 128).transpose(1, 2, 0, 3)).astype(F8)
    wt = np.arange(WIN)
    w_of = (wt % 16) * 16 + wt // 16
    lmask_np = np.ascontiguousarray(
        (w_of.reshape(2, 128)[:, :, None] <= w_of[None, None, :])
        .astype(np.float32).transpose(1, 0, 2)).astype(BF)    # [128, 2, WIN]

    # host-computed local-value summaries
    rows16 = (np.arange(H)[:, None] * 128 + np.arange(16)[None, :]).ravel()
    vl16 = (x2[rows16] @ Wlv).reshape(H, 16, E)
    xsumA = x2.reshape(H, 128, E).sum(axis=1)
    colsumA = xsumA @ Wlv
    colsum16 = vl16.sum(axis=1)
    vbarH = colsumA.reshape(H, 16, 64).sum(axis=1)            # [H, 64]
    tailH = (colsumA - colsum16).reshape(H, 16, 64).sum(axis=1)
    const_row = (vbarH.reshape(E) / S) @ Wp                   # [E]

    # causal cutoff columns for masked tiles
    NT = plan['nt']
    cuts_np = np.zeros((128, NT), np.float32)
    for bidx in plan['order']:
        bl = plan['blocks'][bidx]
        for ti in bl['tiles']:
            if ti['nm']:
                cuts_np[:, ti['mcol']] = 0.5 - ti['cut']

    in_maps = []
    for i in range(NCORES):
        cs = slice(i * 128, (i + 1) * 128)
        wqkv_np = np.stack([
            np.ascontiguousarray(
                W[:, cs].reshape(8, 128, 128).transpose(1, 0, 2))
            for W in (Wq, Wk, Wv)], axis=1).astype(BF)        # [128, 3, 8, 128]
        xTl_np = np.ascontiguousarray(
            x2.T[:, i * 256:(i + 1) * 256]
            .reshape(8, 128, 256).transpose(1, 0, 2)).astype(F8)
        v256_np = np.zeros((128, 2, 2, 65), np.float32)
        tail_np = np.zeros((1, 2, 65), np.float32)
        for hh in range(2):
            hg = 2 * i + hh
            for k2 in range(2):
                wt_ = k2 * 128 + np.arange(128)
                k_true = 16 * (wt_ % 16) + wt_ // 16
                r = k_true // 16
                cpos = k_true % 16
                v256_np[:, hh, k2, 0:64] = vl16[
                    hg, r][np.arange(128)[:, None],
                           (cpos * 64)[:, None] + np.arange(64)[None, :]]
            v256_np[:, hh, :, 64] = 1.0
            tail_np[0, hh, 0:64] = tailH[hg]
            tail_np[0, hh, 64] = S - WIN
        in_maps.append({
            "xT": xT_np,
            "xgT": xgT_np,
            "wqkv": wqkv_np,
            "xTl": xTl_np,
            "wlqk": wlqk_np,
            "lmask": lmask_np,
            "wpr": b(Wp[cs, :]),
            "v256": v256_np.astype(BF),
            "tailv": tail_np.astype(BF),
            "cuts": cuts_np,
        })
    return plan, in_maps, const_row


def assemble(plan, partials, const_row, b_proj, global_attention_mask):
    m = np.asarray(global_attention_mask, np.int64).reshape(S)
    bp = np.asarray(b_proj, np.float32)
    acc = np.zeros((plan['nrows'], E), np.float32)
    for r in partials:
        acc += np.asarray(r["outp"], np.float32)
    out = np.zeros((S, E), np.float32)
    # gathered rows: block b's cols [off, off+w) live at outp rows
    # [roff, roff+w)
    gpad = plan['gpad']
    n1 = plan['n1']
    grows = np.empty(plan['n1p'], np.int64)
    for bidx in plan['order']:
        bl = plan['blocks'][bidx]
        grows[bl['off']:bl['off'] + bl['w']] = np.arange(
            bl['roff'], bl['roff'] + bl['w'])
    out[gpad[:n1]] = acc[grows[:n1]]
    m0 = np.where(m == 0)[0]
    out[m0[m0 >= WIN]] = const_row
    loc_rows = m0[m0 < WIN]
    wperm = (loc_rows % 16) * 16 + loc_rows // 16
    out[loc_rows] = acc[plan['n1p'] + wperm]
    out += bp[None, :]
    return out


def kernel(x, global_attention_mask, W_local_query, W_local_key, W_local_value,
           W_query, W_key, W_value, W_proj, b_proj):
    from concourse.bass_utils import run_bass_kernel_spmd

    plan = make_plan(global_attention_mask)
    key = plan_key(plan)
    if key not in _prog_cache:
        _prog_cache[key] = build_program(plan)
    nc = _prog_cache[key]

    plan, in_maps, const_row = prep_inputs(
        x, global_attention_mask, W_local_query, W_local_key, W_local_value,
        W_query, W_key, W_value, W_proj, plan=plan)
    res = run_bass_kernel_spmd(nc, in_maps, core_ids=list(range(NCORES)))
    out = assemble(plan, res.results, const_row, b_proj,
                   global_attention_mask)
    return out[None].astype(np.float32)
